# revision 8
# baseline (speedup 1.0000x reference)
"""Trainium2 Bass kernel for AlignWaveletFusion (db4, J=4, zero-pad mode).

Sharding: data-parallel over batch B=8 -> one batch element per NeuronCore.
Per core, for all 3 modalities:
  - 4-level analysis DWT along seq as banded matmuls on TensorE
    (seq on partitions, channels on the moving free dim),
  - cross-modal gate GEMMs in bf16 (coefficients transposed on-chip via
    TensorE transpose so the H contraction sits on partitions),
  - sigmoid on ScalarE, gating mul/accumulate on VectorE,
  - 4-level synthesis IDWT as banded matmuls (all 3 target modalities share
    one rhs tile),
  - (recon - x) * weight fused into the last synthesis level.
DMA layout is optimized for few, fat transfers: host pre-pads x, pre-packs
weights so each SBUF partition line is 27.6KB contiguous, and intermediate
coefficients round-trip DRAM in (rows, 3, 768) layout (4.6KB lines).
"""
import os
import sys
import numpy as np
import ml_dtypes

for _p in ("/opt/trn_rl_repo",):
    if _p not in sys.path and os.path.isdir(_p):
        sys.path.insert(0, _p)

import concourse.bass as bass
import concourse.mybir as mybir
from concourse import tile
from concourse.bass_utils import run_bass_kernel_spmd

BF = mybir.dt.bfloat16
F32 = mybir.dt.float32
BF_NP = ml_dtypes.bfloat16

B, S, H, NM, J = 8, 2048, 768, 3, 4
NH = NM * H

DEC_LO = np.array([-0.010597401785069032, 0.0328830116668852, 0.030841381835560764,
                   -0.18703481171909309, -0.027983769416859854, 0.6308807679298589,
                   0.7148465705529157, 0.2303778133088965], dtype=np.float64)
DEC_HI = np.array([-0.2303778133088965, 0.7148465705529157, -0.6308807679298589,
                   -0.027983769416859854, 0.18703481171909309, 0.030841381835560764,
                   -0.0328830116668852, -0.010597401785069032], dtype=np.float64)
H0R, H1R = DEC_LO[::-1], DEC_HI[::-1]
G0F, G1F = DEC_LO, DEC_HI

L_IN = [2048, 1027, 517, 262]
L_OUT = [1027, 517, 262, 134]
# gate coefficient sets: hi1 hi2 hi3 hi4 lo4, at these row offsets in scratch
SET_LEN = [1027, 517, 262, 134, 134]
FOFF = [0, 1027, 1544, 1806, 1940]
NF = 2074
# synthesis steps coarse->fine: (coeff len n, hi set idx, out_len)
SYN = [(134, 3, 262), (262, 2, 518), (517, 1, 1028), (1027, 0, 2048)]

GATE_SLICES = [(0, 512), (512, 512), (1024, 512), (1536, 512), (2048, 256)]
MM_SLICES = [(0, 512), (512, 256)]  # bank-aligned halves of a 768-col psum

XPAD_ROWS = 2080          # 6 zeros + 2048 + tail zeros
# padded cascade-lo buffers: rows = 122*(Wn_next-1)+128 so every window is full
LO_PAD_ROWS = [1104, 616, 372]
NZERO = 112


def _analysis_band():
    A = np.zeros((128, 122), np.float64)
    for k in range(61):
        for t in range(8):
            A[2 * k + t, k] = H0R[t]
            A[2 * k + t, 61 + k] = H1R[t]
    return A


def _synthesis_band():
    Sb = np.zeros((128, 122), np.float64)
    for u in range(61):
        for r in range(4):
            Sb[u + r, 2 * u] += G0F[2 * r + 1]
            Sb[64 + u + r, 2 * u] += G1F[2 * r + 1]
            Sb[u + r, 2 * u + 1] += G0F[2 * r]
            Sb[64 + u + r, 2 * u + 1] += G1F[2 * r]
    return Sb


def _ceil(a, b):
    return -(-a // b)


def _legalize_waits(nc, max_waits=1):
    """Walrus on this stack only accepts one sync-wait command per TPB
    instruction; hoist extras onto standalone EventSemaphore waits (engines
    consume their stream in order, so a preceding same-engine wait is
    semantically identical)."""
    wid = [0]
    for bb in nc.main_func.blocks:
        insts = bb.instructions
        out = []
        changed = False
        for ins in insts:
            si = ins.sync_info
            waits = list(si.on_wait) if si and si.on_wait else []
            if len(waits) > max_waits:
                for w in waits[:-max_waits]:
                    nop = mybir.InstEventSemaphore(
                        name=f"WSPLIT-{wid[0]}", ins=[], outs=[])
                    wid[0] += 1
                    nop.engine = ins.engine
                    nop.sync_info = mybir.SyncInfo(on_wait=[w], on_update=[])
                    out.append(nop)
                ins.sync_info = mybir.SyncInfo(
                    on_wait=waits[-max_waits:],
                    on_update=list(si.on_update) if si.on_update else [])
                changed = True
            out.append(ins)
        if changed:
            insts[:] = out
    return nc


def build_nc(weight, legalize=True):
    nc = bass.Bass()
    x_d = nc.dram_tensor("x", (S, NH), F32, kind="ExternalInput")
    xb_d = nc.dram_tensor("xb", (XPAD_ROWS, NH), BF, kind="ExternalInput")
    wt_d = nc.dram_tensor("wt", (5, NM, 128, 6 * NH), BF, kind="ExternalInput")
    bnd_d = nc.dram_tensor("bands", (128, 5 * 122), BF, kind="ExternalInput")
    idn_d = nc.dram_tensor("ident", (128, 128), BF, kind="ExternalInput")
    zro_d = nc.dram_tensor("zeros", (NZERO, NH), BF, kind="ExternalInput")
    out_d = nc.dram_tensor("out", (S, NH), F32, kind="ExternalOutput")

    with tile.TileContext(nc) as tc:
        import contextlib
        with contextlib.ExitStack() as ctx:
            dram = ctx.enter_context(tc.tile_pool(name="dram", bufs=1, space="DRAM"))
            cpool = ctx.enter_context(tc.tile_pool(name="const", bufs=1))
            winp = ctx.enter_context(tc.tile_pool(name="win", bufs=3))
            cwp = ctx.enter_context(tc.tile_pool(name="cw", bufs=3))
            wtp = ctx.enter_context(tc.tile_pool(name="wts", bufs=3))
            hpp = ctx.enter_context(tc.tile_pool(name="hp", bufs=3))
            ctp = ctx.enter_context(tc.tile_pool(name="ct", bufs=8))
            sgp = ctx.enter_context(tc.tile_pool(name="sg", bufs=2))
            fup = ctx.enter_context(tc.tile_pool(name="fu", bufs=3))
            tmp = ctx.enter_context(tc.tile_pool(name="tm", bufs=2))
            xfp = ctx.enter_context(tc.tile_pool(name="xf", bufs=2))
            ofp = ctx.enter_context(tc.tile_pool(name="of", bufs=2))
            dwt_ps = ctx.enter_context(tc.tile_pool(name="dwtps", bufs=2, space="PSUM"))
            gps = ctx.enter_context(tc.tile_pool(name="gps", bufs=2, space="PSUM"))
            tpp = ctx.enter_context(tc.tile_pool(name="tpps", bufs=2, space="PSUM"))

            # DRAM scratch (axis1 = modality: j for coefficients, m for fused)
            lo_scr = [dram.tile((LO_PAD_ROWS[l], NM, H), BF, name=f"lo{l}",
                                tag=f"lo{l}") for l in range(3)]
            hi_scr = dram.tile((NF, NM, H), BF, name="hiscr", tag="hiscr")
            fu_scr = dram.tile((NF, NM, H), BF, name="fuscr", tag="fuscr")
            synA = dram.tile((518, NM, H), BF, name="synA", tag="synA")
            synB = dram.tile((1028, NM, H), BF, name="synB", tag="synB")

            # constants: bands packed [A | S | Sf0 | Sf1 | Sf2]
            bnd_sb = cpool.tile((128, 5 * 122), BF, name="bands", tag="bands")
            nc.sync.dma_start(bnd_sb[:], bnd_d[:])
            A_sb = bnd_sb[:, 0:122]
            S_sb = bnd_sb[:, 122:244]
            Sf_sb = [bnd_sb[:, 244 + m * 122: 244 + (m + 1) * 122] for m in range(NM)]
            I_sb = cpool.tile((128, 128), BF, name="ident", tag="ident")
            nc.sync.dma_start(I_sb[:], idn_d[:])

            # zero-pad the cascade-lo buffers (head 6 rows + tail)
            for l in range(3):
                head, n = 6, L_OUT[l]
                nc.gpsimd.dma_start(
                    lo_scr[l][0:head].rearrange("r j h -> r (j h)"),
                    zro_d[0:head])
                tail = LO_PAD_ROWS[l] - head - n
                nc.gpsimd.dma_start(
                    lo_scr[l][head + n:].rearrange("r j h -> r (j h)"),
                    zro_d[0:tail])

            # ---------------- gating for one coefficient set ----------------
            def gate_set(s):
                n = SET_LEN[s]
                nch = _ceil(n, 128)
                wts = []
                for j in range(NM):
                    wt_t = wtp.tile((128, 6 * NH), BF, name="wt", tag="wt")
                    nc.sync.dma_start(wt_t[:], wt_d[s, j])
                    wts.append(wt_t)
                for c in range(nch):
                    rows = min(128, n - 128 * c)
                    hp = hpp.tile((128, NH), BF, name="hp", tag="hp")
                    if rows < 128:
                        nc.vector.memset(hp[:], 0.0)
                    nc.sync.dma_start(
                        hp[0:rows],
                        hi_scr[FOFF[s] + 128 * c: FOFF[s] + 128 * c + rows]
                        .rearrange("r j h -> r (j h)"))
                    fu = fup.tile((128, NH), BF, name="fu", tag="fu")
                    for j in range(NM):
                        cts = []
                        for hc in range(6):
                            tp = tpp.tile((128, 128), BF, name="tp", tag="tp")
                            nc.tensor.transpose(
                                tp[:], hp[:, j * H + hc * 128: j * H + (hc + 1) * 128],
                                I_sb[:])
                            ct = ctp.tile((128, 128), BF, name="ct", tag="ct")
                            nc.any.tensor_copy(ct[:], tp[:])
                            cts.append(ct)
                        sg = sgp.tile((128, NH), BF, name="sg", tag="sg")
                        for off, ln in GATE_SLICES:
                            gp = gps.tile((128, 512), F32, name="gp", tag="gp")
                            for hc in range(6):
                                nc.tensor.matmul(
                                    gp[:, 0:ln], cts[hc][:],
                                    wts[j][:, hc * NH + off: hc * NH + off + ln],
                                    start=(hc == 0), stop=(hc == 5))
                            nc.scalar.activation(
                                sg[:, off:off + ln], gp[:, 0:ln],
                                mybir.ActivationFunctionType.Sigmoid)
                        for m in range(NM):
                            if j == 0:
                                nc.vector.tensor_mul(
                                    fu[:, m * H:(m + 1) * H],
                                    sg[:, m * H:(m + 1) * H], hp[:, 0:H])
                            else:
                                tm = tmp.tile((128, H), BF, name="tm", tag="tm")
                                nc.vector.tensor_mul(
                                    tm[:], sg[:, m * H:(m + 1) * H],
                                    hp[:, j * H:(j + 1) * H])
                                nc.vector.tensor_add(
                                    fu[:, m * H:(m + 1) * H],
                                    fu[:, m * H:(m + 1) * H], tm[:])
                    nc.gpsimd.dma_start(
                        fu_scr[FOFF[s] + 128 * c: FOFF[s] + 128 * c + rows]
                        .rearrange("r m h -> r (m h)"),
                        fu[0:rows])

            # ---------------- analysis + gating, level by level ----------------
            for lvl in range(4):
                n_out = L_OUT[lvl]
                Wn = _ceil(n_out, 61)
                for w in range(Wn):
                    xw = winp.tile((128, NH), BF, name="xw", tag="win")
                    if lvl == 0:
                        nc.sync.dma_start(xw[:], xb_d[122 * w: 122 * w + 128])
                    else:
                        nc.sync.dma_start(
                            xw[:], lo_scr[lvl - 1][122 * w: 122 * w + 128]
                            .rearrange("r j h -> r (j h)"))
                    cw = cwp.tile((122, NH), BF, name="cw", tag="cw")
                    for j in range(NM):
                        ps = dwt_ps.tile((122, H), F32, name="dps", tag="dps")
                        for off, ln in MM_SLICES:
                            nc.tensor.matmul(ps[:, off:off + ln], A_sb,
                                             xw[:, j * H + off: j * H + off + ln],
                                             start=True, stop=True)
                        nc.any.tensor_copy(cw[:, j * H:(j + 1) * H], ps[:])
                    kmax = min(61, n_out - 61 * w)
                    if lvl < 3:
                        nc.gpsimd.dma_start(
                            lo_scr[lvl][6 + 61 * w: 6 + 61 * w + kmax]
                            .rearrange("r j h -> r (j h)"),
                            cw[0:kmax])
                    nc.gpsimd.dma_start(
                        hi_scr[FOFF[lvl] + 61 * w: FOFF[lvl] + 61 * w + kmax]
                        .rearrange("r j h -> r (j h)"),
                        cw[61:61 + kmax])
                    if lvl == 3:
                        nc.gpsimd.dma_start(
                            hi_scr[FOFF[4] + 61 * w: FOFF[4] + 61 * w + kmax]
                            .rearrange("r j h -> r (j h)"),
                            cw[0:kmax])
                gate_set(lvl)
                if lvl == 3:
                    gate_set(4)

            # ---------------- synthesis (all 3 modalities per window) ----------
            for step, (n, hi_s, out_len) in enumerate(SYN):
                last = step == len(SYN) - 1
                if step == 0:
                    lo_t, lo_off = fu_scr, FOFF[4]
                elif step == 2:
                    lo_t, lo_off = synA, 0
                else:
                    lo_t, lo_off = synB, 0
                dst = synA if step == 1 else synB
                Wv = _ceil(out_len, 122)
                for v in range(Wv):
                    q0 = 61 * v
                    qv = min(64, n - q0)
                    rb = winp.tile((128, NH), BF, name="rb", tag="win")
                    if qv < 64:
                        nc.vector.memset(rb[:], 0.0)
                    nc.sync.dma_start(rb[0:qv],
                                      lo_t[lo_off + q0: lo_off + q0 + qv]
                                      .rearrange("r m h -> r (m h)"))
                    nc.sync.dma_start(rb[64:64 + qv],
                                      fu_scr[FOFF[hi_s] + q0: FOFF[hi_s] + q0 + qv]
                                      .rearrange("r m h -> r (m h)"))
                    kmax = min(122, out_len - 122 * v)
                    if not last:
                        ob = cwp.tile((122, NH), BF, name="ob", tag="cw")
                        for m in range(NM):
                            ps = dwt_ps.tile((122, H), F32, name="dps", tag="dps")
                            for off, ln in MM_SLICES:
                                nc.tensor.matmul(ps[:, off:off + ln], S_sb,
                                                 rb[:, m * H + off: m * H + off + ln],
                                                 start=True, stop=True)
                            nc.any.tensor_copy(ob[:, m * H:(m + 1) * H], ps[:])
                        nc.gpsimd.dma_start(
                            dst[122 * v: 122 * v + kmax].rearrange("r m h -> r (m h)"),
                            ob[0:kmax])
                    else:
                        xf = xfp.tile((122, NH), F32, name="xf", tag="xf")
                        nc.sync.dma_start(xf[0:kmax], x_d[122 * v: 122 * v + kmax])
                        of = ofp.tile((122, NH), F32, name="of", tag="of")
                        for m in range(NM):
                            ps = dwt_ps.tile((122, H), F32, name="dps", tag="dps")
                            for off, ln in MM_SLICES:
                                nc.tensor.matmul(ps[:, off:off + ln], Sf_sb[m],
                                                 rb[:, m * H + off: m * H + off + ln],
                                                 start=True, stop=True)
                            nc.vector.scalar_tensor_tensor(
                                of[0:kmax, m * H:(m + 1) * H],
                                xf[0:kmax, m * H:(m + 1) * H], -float(weight[m]),
                                ps[0:kmax, :], mybir.AluOpType.mult,
                                mybir.AluOpType.add)
                        nc.gpsimd.dma_start(out_d[122 * v: 122 * v + kmax],
                                            of[0:kmax])
    if legalize:
        _legalize_waits(nc)
    return nc


def host_inputs(low_w, high_w, weight):
    """Pre-transpose/pack gate weights and band constants (bf16)."""
    # gate rhs: wtf[s, j, h, m*H+g] = W_s[m, j, g, h]
    wtf = np.empty((5, NM, H, NH), dtype=BF_NP)
    for k in range(J):
        wtf[k] = high_w[:, k].transpose(1, 3, 0, 2).reshape(NM, H, NH).astype(BF_NP)
    wtf[4] = low_w.transpose(1, 3, 0, 2).reshape(NM, H, NH).astype(BF_NP)
    # repack so each SBUF partition line (6 hc blocks) is contiguous: 27.6KB
    wt = np.ascontiguousarray(
        wtf.reshape(5, NM, 6, 128, NH).transpose(0, 1, 3, 2, 4)
        .reshape(5, NM, 128, 6 * NH))

    A = _analysis_band()
    Sb = _synthesis_band()
    bands = np.empty((128, 5 * 122), dtype=BF_NP)
    bands[:, 0:122] = A.astype(BF_NP)
    bands[:, 122:244] = Sb.astype(BF_NP)
    for m in range(NM):
        bands[:, 244 + m * 122: 244 + (m + 1) * 122] = \
            (Sb * float(weight[m])).astype(BF_NP)
    ident = np.eye(128, dtype=BF_NP)
    zeros = np.zeros((NZERO, NH), dtype=BF_NP)
    return wt, bands, ident, zeros


def make_in_maps(x, low_w, high_w, weight):
    wt, bands, ident, zeros = host_inputs(low_w, high_w, weight)
    xp = np.zeros((B, XPAD_ROWS, NH), dtype=BF_NP)
    xp[:, 6:6 + S] = x.astype(BF_NP)
    return [{"x": np.ascontiguousarray(x[b]), "xb": xp[b], "wt": wt,
             "bands": bands, "ident": ident, "zeros": zeros} for b in range(B)]


_CACHE = {}


def kernel(x, low_w, high_w, weight):
    x = np.asarray(x, np.float32)
    low_w = np.asarray(low_w, np.float32)
    high_w = np.asarray(high_w, np.float32)
    weight = np.asarray(weight, np.float32)

    key = weight.tobytes()
    if key not in _CACHE:
        _CACHE[key] = build_nc(weight)
    nc = _CACHE[key]

    in_maps = make_in_maps(x, low_w, high_w, weight)
    res = run_bass_kernel_spmd(nc, in_maps, list(range(B))).results
    # out rows are (S, [m0|m1|m2]*H) -> (3, B, S, H)
    out = np.stack([res[b]["out"].reshape(S, NM, H).transpose(1, 0, 2)
                    for b in range(B)], axis=1)
    return np.ascontiguousarray(out.astype(np.float32))
